# revision 47
# baseline (speedup 1.0000x reference)
"""Trainium2 Bass kernel for the external-knowledge memory network problem.

Math (after dead-code elimination of the reference):
  raw_t[b,m,:] = sum_s C[t][story[b,m,s]] + dh_add(b,m)        t = 0,1,2
  eA_t = lrelu(concat([raw_t, tf]) @ wA1 + bA1) @ wA2 + bA2
  uq_0 = query_vector
  hop h: logits_h = (eA_h * gp) . uq_h ; p_h = softmax(logits_h)
         uq_{h+1} = uq_h + (eA_{h+1} * gp)^T p_h               (h = 0,1)
  returns (p_2, logits_2)

The u / logit / p recurrence inside load_ent_memory and table C[3] (eC_2)
never reach the output, so they are skipped entirely.

Sharding: data-parallel over batch, 2 examples per core x 8 cores.
Gathers: fp16 concatenated table rows [C0[v] | C1[v] | C2[v]] (768 B)
via the custom SWDGE vector gather (nc.gpsimd.dma_gather, int16 index
vector wrapped into 16 partitions and replicated across the 8 Q7 core
groups).  SWDGE costs ~1us fixed per instruction + ~0.34ns/descriptor,
and dma_gather tops out at 1024 indices/instruction (larger counts or
transpose=True crash the device), so each (example, quarter) issues 3
gathers of 1024 rows into an 18KB/partition staging tile.  Gathered
row i lands at partition i%128, chunk i//128; indices are ordered so
the staging layout is u-major (token-slice-major), matching the DVE
6-way token sum + dh addition (dh is a host-shifted dense tensor on
the HWDGE queue).  lrelu is one fused scalar_tensor_tensor on DVE.

NOTE: plain indirect_dma_start only honors ONE index per partition per
instruction (the INDIRECT1D ISA has no per-channel index count); a
multi-column offset AP silently degrades to a contiguous block fetch.
"""

import sys

if "/opt/trn_rl_repo" not in sys.path:
    sys.path.insert(0, "/opt/trn_rl_repo")

import numpy as np

B, M, S, E, LC, V = 16, 2048, 6, 128, 512, 32000
NCORES = 8
BL = B // NCORES          # examples per core
NT = 3                    # tables used (0..2)
NJM = M // 128            # 16 column-chunks of 128 m's
NQ = 4                    # gather quarters
JMQ = NJM // NQ           # 4 jm per quarter
NG = 3                    # dma_gather calls per quarter (1024 idxs each)
GN = 1024                 # indices per dma_gather
GW = GN // 16             # idx columns per gather (wrapped in 16 partitions)
IW = NQ * NG * GW         # idx columns per example (768)

_CACHE = {}


def _build_nc():
    import concourse.bacc as bacc
    import concourse.tile as tile
    from concourse import bass, mybir

    fp16 = mybir.dt.float16
    f32 = mybir.dt.float32
    i16 = mybir.dt.int16
    LR = mybir.ActivationFunctionType.Lrelu
    CP = mybir.ActivationFunctionType.Copy
    IDF = mybir.ActivationFunctionType.Identity
    EXP = mybir.ActivationFunctionType.Exp
    ADD = mybir.AluOpType.add

    nc = bacc.Bacc("TRN2", target_bir_lowering=False, num_swdge_queues=4)

    ccx = nc.dram_tensor("ccx", [V, NT * E], fp16, kind="ExternalInput")
    dhs = nc.dram_tensor("dhs", [BL, 128, NJM * NT * E], fp16, kind="ExternalInput")
    # int16 index vectors for dma_gather: per (example, quarter, gather) a
    # [128, GW] block, wrapped in 16 partitions and replicated x8 vertically
    idx = nc.dram_tensor("idx", [BL, 128, IW], i16, kind="ExternalInput")
    w1a = nc.dram_tensor("w1a", [E, E], fp16, kind="ExternalInput")
    w1b = nc.dram_tensor("w1b", [E, E], fp16, kind="ExternalInput")
    w2 = nc.dram_tensor("w2", [E, E], fp16, kind="ExternalInput")
    ba1 = nc.dram_tensor("ba1", [E, 1], f32, kind="ExternalInput")
    ba2 = nc.dram_tensor("ba2", [E, 1], f32, kind="ExternalInput")
    tf = nc.dram_tensor("tf", [BL, E], f32, kind="ExternalInput")
    qv = nc.dram_tensor("qv", [BL, E], f32, kind="ExternalInput")
    gpt = nc.dram_tensor("gpt", [BL, 128, NJM], f32, kind="ExternalInput")
    id16d = nc.dram_tensor("id16", [128, 128], fp16, kind="ExternalInput")
    id32d = nc.dram_tensor("id32", [128, 128], f32, kind="ExternalInput")
    onesd = nc.dram_tensor("ones", [128, 128], f32, kind="ExternalInput")
    outp = nc.dram_tensor("outp", [BL, 128, NJM], f32, kind="ExternalOutput")
    outl = nc.dram_tensor("outl", [BL, 128, NJM], f32, kind="ExternalOutput")

    with tile.TileContext(nc) as tc:
        with (
            tc.tile_pool(name="const", bufs=1) as const,
            tc.tile_pool(name="big", bufs=1) as big,
            tc.tile_pool(name="work", bufs=4) as work,
            tc.tile_pool(name="g", bufs=4) as gpool,
            tc.tile_pool(name="pt", bufs=2, space="PSUM") as pst,
            tc.tile_pool(name="pmm", bufs=2, space="PSUM") as pmm,
            tc.tile_pool(name="psm", bufs=2, space="PSUM") as psm,
        ):
            # ---- constants / small inputs ----
            idx_s = [const.tile([128, IW], i16, name=f"idx{b}", tag=f"idx{b}") for b in range(BL)]
            for b in range(BL):
                nc.sync.dma_start(out=idx_s[b][:], in_=idx[b])
            # prefetch all dh data during the boot window, off the contended
            # gather-phase DMA path (3.1MB that would otherwise fight the
            # table-row drain for SDMA bandwidth)
            dhall = [
                const.tile([128, NJM * NT * E], fp16, name=f"dh{b}", tag=f"dh{b}")
                for b in range(BL)
            ]
            for b in range(BL):
                nc.sync.dma_start(out=dhall[b][:], in_=dhs[b])
            id16 = const.tile([128, 128], fp16)
            nc.sync.dma_start(out=id16[:], in_=id16d[:])
            id32 = const.tile([128, 128], f32)
            nc.sync.dma_start(out=id32[:], in_=id32d[:])
            ones_sq = const.tile([128, 128], f32)
            nc.sync.dma_start(out=ones_sq[:], in_=onesd[:])

            w1a_s = const.tile([E, E], fp16)
            nc.sync.dma_start(out=w1a_s[:], in_=w1a[:])
            w1b_s = const.tile([E, E], fp16)
            nc.sync.dma_start(out=w1b_s[:], in_=w1b[:])
            w2_s = const.tile([E, E], fp16)
            nc.sync.dma_start(out=w2_s[:], in_=w2[:])
            ba1_s = const.tile([E, 1], f32)
            nc.sync.dma_start(out=ba1_s[:], in_=ba1[:])
            ba2_s = const.tile([E, 1], f32)
            nc.sync.dma_start(out=ba2_s[:], in_=ba2[:])
            tf_s = const.tile([BL, E], f32)
            nc.sync.dma_start(out=tf_s[:], in_=tf[:])
            qv_s = const.tile([BL, E], f32)
            nc.sync.dma_start(out=qv_s[:], in_=qv[:])
            gpt_s = [const.tile([128, NJM], f32, name=f"gpt{b}", tag=f"gpt{b}") for b in range(BL)]
            for b in range(BL):
                nc.sync.dma_start(out=gpt_s[b][:], in_=gpt[b])
            # tf/qv transposed to columns; biasA = w1b.T @ tfT + ba1
            ptf = psm.tile([128, 128], f32, tag="sm")
            nc.tensor.transpose(ptf[:, :BL], tf_s[:], id32[:BL, :BL])
            tfT16 = const.tile([128, BL], fp16)
            nc.scalar.copy(out=tfT16[:], in_=ptf[:, :BL])
            pqv = psm.tile([128, 128], f32, tag="sm")
            nc.tensor.transpose(pqv[:, :BL], qv_s[:], id32[:BL, :BL])
            uq_s = const.tile([128, BL], f32)
            nc.vector.tensor_copy(out=uq_s[:], in_=pqv[:, :BL])

            pba = psm.tile([128, 128], f32, tag="sm")
            nc.tensor.matmul(pba[:, :BL], lhsT=w1b_s[:], rhs=tfT16[:], start=True, stop=True)
            biasA = const.tile([128, BL], f32)
            nc.vector.tensor_tensor(
                out=biasA[:], in0=pba[:, :BL],
                in1=ba1_s[:, :1].to_broadcast([128, BL]), op=ADD,
            )

            # ---- persistent big tiles ----
            rawT = [big.tile([128, NT * M], fp16, name=f"rt{b}", tag=f"rt{b}") for b in range(BL)]
            eAT = [big.tile([128, NT * M], fp16, name=f"ea{b}", tag=f"ea{b}") for b in range(BL)]
            eAM = [big.tile([128, 2 * M], fp16, name=f"em{b}", tag=f"em{b}") for b in range(BL)]

            # ---- per (example, quarter): gathers, dh add, transpose, MLP ----
            W = NT * E
            QB = JMQ * W  # quarter block: 1536 elems
            for b in range(BL):
                for q in range(NQ):
                    # 3 batched vector gathers per (example, quarter), 1024
                    # rows each.  Gather gi covers token slices u=2gi,2gi+1;
                    # row i lands at partition i%128, chunk i//128, and the
                    # host orders indices so chunk c = (u%2)*JMQ + j, giving
                    # the u-major staging layout g[p, u*QB + j*W] the sums
                    # below expect.
                    g = gpool.tile([128, S * QB], fp16, tag="g")
                    for gi in range(NG):
                        co = (q * NG + gi) * GW
                        # round-robin across the 4 SWDGE queues: each queue is
                        # served by its own Q7 core pair, so descriptor
                        # generation for different queues runs concurrently
                        nc.gpsimd.dma_gather(
                            g[:, gi * 2 * QB:(gi + 1) * 2 * QB].rearrange(
                                "p (c w) -> p c w", w=W
                            ),
                            ccx[:],
                            idx_s[b][:, co:co + GW],
                            GN,
                            GN,
                            W,
                            queue_num=((b * NQ + q) * NG + gi) % 4,
                        )
                    rawq = work.tile([128, QB], fp16, tag="rawq", bufs=2)
                    rcq = rawq[:]
                    nc.vector.tensor_add(
                        out=rcq, in0=g[:, 0:QB],
                        in1=dhall[b][:, q * QB:(q + 1) * QB],
                    )
                    for u in range(1, S):
                        nc.vector.tensor_add(
                            out=rcq, in0=rcq, in1=g[:, u * QB:(u + 1) * QB]
                        )
                    # transpose [m, t, e] -> rawT [e, t*M + m]
                    for j in range(JMQ):
                        jm = q * JMQ + j
                        pt = pst.tile([128, NT * E], fp16, tag="pt")
                        for t in range(NT):
                            nc.tensor.transpose(
                                pt[:, t * E:(t + 1) * E],
                                rawq[:, (j * NT + t) * E:(j * NT + t + 1) * E],
                                id16[:],
                            )
                        nc.scalar.copy(
                            out=rawT[b][:].rearrange("p (t m) -> p t m", t=NT)[
                                :, :, jm * E:(jm + 1) * E
                            ],
                            in_=pt[:].rearrange("p (t e) -> p t e", t=NT),
                        )
                    # MLP on this quarter's 512 columns of each table
                    for t in range(NT):
                        c0 = t * M + q * 512
                        p1 = pmm.tile([128, 512], f32, tag="p1")
                        nc.tensor.matmul(
                            p1[:], lhsT=w1a_s[:], rhs=rawT[b][:, c0:c0 + 512],
                            start=True, stop=True,
                        )
                        t1 = work.tile([128, 512], fp16, tag="t1")
                        nc.scalar.activation(
                            out=t1[:], in_=p1[:], func=IDF,
                            bias=biasA[:, b:b + 1],
                        )
                        h1 = work.tile([128, 512], fp16, tag="h1")
                        nc.vector.scalar_tensor_tensor(
                            out=h1[:], in0=t1[:], scalar=0.1, in1=t1[:],
                            op0=mybir.AluOpType.mult, op1=mybir.AluOpType.max,
                        )
                        p2 = pmm.tile([128, 512], f32, tag="p2")
                        nc.tensor.matmul(
                            p2[:], lhsT=w2_s[:], rhs=h1[:], start=True, stop=True
                        )
                        nc.scalar.activation(
                            out=eAT[b][:, c0:c0 + 512], in_=p2[:], func=IDF,
                            bias=ba2_s[:, :1],
                        )

                    # transpose eA tables 1,2 back to [m, e] layout for uq updates
                    for j in range(JMQ):
                        jm = q * JMQ + j
                        pt2 = pst.tile([128, NT * E], fp16, tag="pt")
                        for ti in range(2):
                            nc.tensor.transpose(
                                pt2[:, ti * E:(ti + 1) * E],
                                eAT[b][:, (ti + 1) * M + jm * E:(ti + 1) * M + (jm + 1) * E],
                                id16[:],
                            )
                        if b == 0:
                            nc.scalar.copy(
                                out=eAM[b][:].rearrange("p (t m) -> p t m", t=2)[
                                    :, :, jm * E:(jm + 1) * E
                                ],
                                in_=pt2[:, :2 * E].rearrange("p (t e) -> p t e", t=2),
                            )
                        else:
                            # b=1's post-gather chain is the tail; its PSUM->
                            # SBUF copies go to DVE to offload the saturated
                            # Scalar engine there
                            nc.vector.tensor_copy(
                                out=eAM[b][:].rearrange("p (t m) -> p t m", t=2)[
                                    :, :, jm * E:(jm + 1) * E
                                ],
                                in_=pt2[:, :2 * E].rearrange("p (t e) -> p t e", t=2),
                            )

                # ---- hop chain for this example ----
                for h in range(NT):
                    logit = work.tile([128, NJM], f32, tag="lg")
                    uq16 = work.tile([128, 1], fp16, tag="uq16")
                    nc.vector.tensor_copy(out=uq16[:], in_=uq_s[:, b:b + 1])
                    pl = psm.tile([128, 128], f32, tag="sm")
                    for jm in range(NJM):
                        nc.tensor.matmul(
                            pl[:, jm:jm + 1],
                            lhsT=eAT[b][:, h * M + jm * E:h * M + (jm + 1) * E],
                            rhs=uq16[:], start=True, stop=True,
                        )
                    nc.vector.tensor_mul(out=logit[:], in0=pl[:, :NJM], in1=gpt_s[b][:])
                    es = work.tile([128, NJM], f32, tag="es")
                    rs = work.tile([128, 1], f32, tag="rs")
                    nc.scalar.activation(out=es[:], in_=logit[:], func=EXP, accum_out=rs[:])
                    # all-partition sum, broadcast to every partition, in one
                    # matmul: ones.T @ rs
                    pbc = psm.tile([128, 128], f32, tag="sm")
                    nc.tensor.matmul(
                        pbc[:, :1], lhsT=ones_sq[:], rhs=rs[:], start=True, stop=True
                    )
                    rcb = work.tile([128, 1], f32, tag="rcb")
                    nc.vector.reciprocal(out=rcb[:], in_=pbc[:, :1])
                    if h == 2:
                        nc.sync.dma_start(out=outl[b], in_=logit[:])
                        pfin = work.tile([128, NJM], f32, tag="pfin")
                        nc.vector.tensor_scalar_mul(
                            out=pfin[:], in0=es[:], scalar1=rcb[:, :1]
                        )
                        nc.sync.dma_start(out=outp[b], in_=pfin[:])
                    else:
                        pn = work.tile([128, NJM], f32, tag="pn")
                        nc.vector.tensor_scalar_mul(
                            out=pn[:], in0=es[:], scalar1=rcb[:, :1]
                        )
                        pw = work.tile([128, NJM], fp16, tag="pw")
                        nc.vector.tensor_mul(out=pw[:], in0=pn[:], in1=gpt_s[b][:])
                        pv = psm.tile([128, 128], f32, tag="sm")
                        for jm in range(NJM):
                            nc.tensor.matmul(
                                pv[:, :1],
                                lhsT=eAM[b][:, h * M + jm * E:h * M + (jm + 1) * E],
                                rhs=pw[:, jm:jm + 1],
                                start=(jm == 0), stop=(jm == NJM - 1),
                            )
                        nc.vector.tensor_add(
                            out=uq_s[:, b:b + 1], in0=uq_s[:, b:b + 1], in1=pv[:, :1]
                        )

    nc.finalize()
    return nc


def _get_nc():
    if "nc" not in _CACHE:
        _CACHE["nc"] = _build_nc()
    return _CACHE["nc"]


def _prep_core(c, story, kb_len, conv_len, dh, tfh, qvec, gp, C3, w1af, w1bf, w2f, ba1f, ba2f):
    """Build the per-core input map."""
    idx = np.empty((BL, 128, IW), np.int16)
    dhs = np.zeros((BL, 128, NJM * NT * E), np.float16)
    for b in range(BL):
        bg = c * BL + b
        # per (q, gi): flat index i = ((s%2)*JMQ + j)*128 + p for token
        # (m=(q*JMQ+j)*128+p, s=2gi+s%2); wrap into 16 partitions
        # (wrapped[p16, col] = flat[col*16+p16]) and replicate x8
        st4 = story[bg].astype(np.int16).reshape(NQ, JMQ, 128, S)
        blocks = []
        for q in range(NQ):
            for gi in range(NG):
                flat = st4[q, :, :, 2 * gi:2 * gi + 2].transpose(2, 0, 1).reshape(GN)
                blocks.append(np.tile(flat.reshape(GW, 16).T, (8, 1)))
        idx[b] = np.concatenate(blocks, axis=1)
        kb = int(kb_len[bg])
        cl = int(conv_len[bg])
        dhm = np.zeros((M, NT * E), np.float16)
        dhm[kb:kb + cl] = np.tile(dh[bg, :cl].astype(np.float16), (1, NT))
        # [m, w] -> [p, jm*W + w] with m = jm*128 + p
        dhs[b] = dhm.reshape(NJM, 128, NT * E).transpose(1, 0, 2).reshape(128, NJM * NT * E)

    sl = slice(c * BL, (c + 1) * BL)
    gpt = np.ascontiguousarray(
        gp[sl].reshape(BL, NJM, 128).transpose(0, 2, 1)
    ).astype(np.float32)
    return {
        "ccx": C3,
        "idx": idx,
        "dhs": dhs,
        "w1a": w1af,
        "w1b": w1bf,
        "w2": w2f,
        "ba1": ba1f,
        "ba2": ba2f,
        "id16": np.eye(128, dtype=np.float16),
        "id32": np.eye(128, dtype=np.float32),
        "ones": np.ones((128, 128), np.float32),
        "tf": np.ascontiguousarray(tfh[sl]).astype(np.float32),
        "qv": np.ascontiguousarray(qvec[sl]).astype(np.float32),
        "gpt": gpt,
    }


def make_in_maps(story, kb_len, conv_len, dh_outputs, tf_hidden, query_vector,
                 global_pointer, C):
    story = np.asarray(story)
    kb_len = np.asarray(kb_len)
    conv_len = np.asarray(conv_len)
    dh = np.asarray(dh_outputs, dtype=np.float32)
    tfh = np.asarray(tf_hidden, dtype=np.float32)
    qvec = np.asarray(query_vector, dtype=np.float32)
    gp = np.asarray(global_pointer, dtype=np.float32)
    C = np.asarray(C, dtype=np.float32)

    C3 = np.ascontiguousarray(C[:NT].transpose(1, 0, 2).reshape(V, NT * E)).astype(
        np.float16
    )
    wA1 = np.asarray(_CACHE["wA1"], np.float32)
    wA2 = np.asarray(_CACHE["wA2"], np.float32)
    w1af = np.ascontiguousarray(wA1[:E]).astype(np.float16)
    w1bf = np.ascontiguousarray(wA1[E:]).astype(np.float16)
    w2f = wA2.astype(np.float16)
    ba1f = np.asarray(_CACHE["bA1"], np.float32).reshape(E, 1)
    ba2f = np.asarray(_CACHE["bA2"], np.float32).reshape(E, 1)

    return [
        _prep_core(c, story, kb_len, conv_len, dh, tfh, qvec, gp, C3,
                   w1af, w1bf, w2f, ba1f, ba2f)
        for c in range(NCORES)
    ]


def _unshard(results):
    prob = np.empty((B, M), np.float32)
    logits = np.empty((B, M), np.float32)
    for c in range(NCORES):
        for b in range(BL):
            bg = c * BL + b
            prob[bg] = results[c]["outp"][b].T.reshape(M)
            logits[bg] = results[c]["outl"][b].T.reshape(M)
    return prob, logits


def kernel(story, kb_len, conv_len, hidden, dh_outputs, tf_hidden, query_vector,
           global_pointer, C, wA1, bA1, wA2, bA2, wC1, bC1, wC2, bC2, wf, bf,
           **_unused):
    from concourse.bass_utils import run_bass_kernel_spmd

    _CACHE["wA1"], _CACHE["wA2"] = wA1, wA2
    _CACHE["bA1"], _CACHE["bA2"] = bA1, bA2
    nc = _get_nc()
    in_maps = make_in_maps(story, kb_len, conv_len, dh_outputs, tf_hidden,
                           query_vector, global_pointer, C)
    res = run_bass_kernel_spmd(nc, in_maps, list(range(NCORES)))
    return _unshard(res.results)

